# revision 1
# baseline (speedup 1.0000x reference)
"""DeformAtten2D kernel for 8 Trainium2 NeuronCores.

Strategy (per spec sharding_hint): data-parallel over batch B=16 across the
8 cores (2 images per core); the small 64x64 weights are replicated. The whole
forward pass executes on-device via PJRT on the axon-tunneled NeuronCores.

Hardcoded problem shape: x (16, 128, 128, 64) fp32, C == n_heads == 64,
KS = 5, offset_range_factor = 5.
"""

import numpy as np

B, H, W, C = 16, 128, 128, 64
KS = 5
NCORES = 8

_compiled = None


def _build():
    global _compiled
    if _compiled is not None:
        return _compiled

    import jax
    import jax.numpy as jnp
    from jax import lax

    def _conv1x1(x, w, b=None):
        # x: (b,C,H,W), w: (Cout,Cin)
        y = jnp.einsum('bchw,oc->bohw', x, w)
        return y if b is None else y + b[None, :, None, None]

    def _grid_sample(img, gx, gy):
        b, c, h, w = img.shape
        ix = ((gx + 1.0) * w - 1.0) * 0.5
        iy = ((gy + 1.0) * h - 1.0) * 0.5
        ix0 = jnp.floor(ix)
        iy0 = jnp.floor(iy)
        ix1, iy1 = ix0 + 1.0, iy0 + 1.0
        wx1 = ix - ix0
        wx0 = 1.0 - wx1
        wy1 = iy - iy0
        wy0 = 1.0 - wy1
        flat = img.reshape(b, c, h * w)

        def gather(jx, jy):
            valid = (jx >= 0) & (jx <= w - 1) & (jy >= 0) & (jy <= h - 1)
            jxc = jnp.clip(jx, 0, w - 1).astype(jnp.int32)
            jyc = jnp.clip(jy, 0, h - 1).astype(jnp.int32)
            idx = (jyc * w + jxc).reshape(b, 1, h * w)
            vals = jnp.take_along_axis(flat, idx, axis=2).reshape(b, c, h, w)
            return vals * valid.astype(img.dtype)[:, None]

        return (gather(ix0, iy0) * (wx0 * wy0)[:, None]
                + gather(ix1, iy0) * (wx1 * wy0)[:, None]
                + gather(ix0, iy1) * (wx0 * wy1)[:, None]
                + gather(ix1, iy1) * (wx1 * wy1)[:, None])

    def fwd(x, wq, bq, wk, bk, wv, bv, w_off1, b_off1, w_off2, w_out, b_out, rel_bias):
        # x: (2, H, W, C) local batch shard
        b, h, w, c = x.shape
        scale_factor = c ** (-0.5)
        xc = jnp.transpose(x, (0, 3, 1, 2))
        q = _conv1x1(xc, wq, bq)
        off = lax.conv_general_dilated(
            q, w_off1, (1, 1), ((KS // 2, KS // 2), (KS // 2, KS // 2)),
            dimension_numbers=('NCHW', 'OIHW', 'NCHW'))
        off = off + b_off1[None, :, None, None]
        off = jnp.einsum('bchw,oc->bohw', off, w_off2)
        off = jnp.tanh(off) * float(KS)
        vx = jnp.arange(w, dtype=x.dtype)[None, None, :] + off[:, 0]
        vy = jnp.arange(h, dtype=x.dtype)[None, :, None] + off[:, 1]
        gxn = 2.0 * vx / max(h - 1, 1) - 1.0
        gyn = 2.0 * vy / max(w - 1, 1) - 1.0
        x_sampled = _grid_sample(xc, gxn, gyn)
        k = _conv1x1(x_sampled, wk, bk).reshape(b * c, h, w)
        v = (_conv1x1(x_sampled, wv, bv) + rel_bias).reshape(b * c, h, w)
        qh = q.reshape(b * c, h, w)
        attn = jax.nn.softmax(jnp.einsum('bid,bjd->bij', qh, k) * scale_factor, axis=-1)
        out = jnp.einsum('bij,bjd->bid', attn, v).reshape(b, h, w, c)
        return jnp.einsum('bhwc,oc->bhwo', out, w_out) + b_out

    wnames = ['wq', 'bq', 'wk', 'bk', 'wv', 'bv', 'w_off1', 'b_off1',
              'w_off2', 'w_out', 'b_out', 'rel_bias']
    pm = jax.pmap(fwd, axis_name='i',
                  in_axes=(0,) + (None,) * len(wnames),
                  devices=jax.devices()[:NCORES])
    _compiled = (jax, jnp, pm, wnames)
    return _compiled


def kernel(**inputs) -> np.ndarray:
    jax, jnp, pm, wnames = _build()
    x = np.asarray(inputs['x'], dtype=np.float32)
    xs = x.reshape(NCORES, B // NCORES, H, W, C)
    args = [np.asarray(inputs[n], dtype=np.float32) for n in wnames]
    out = pm(xs, *args)
    out = np.asarray(out, dtype=np.float32).reshape(B, H, W, C)
    return out
